# revision 1
# baseline (speedup 1.0000x reference)
"""BoxConv2d Trainium2 kernel.

Reference computes, per (c, f) box and batch b:
    out[b, c*FN+f, i, j] = integral of x[b, c] over the continuous window
        rows [i + x_min, i + x_max + 1) x cols [j + y_min, j + y_max + 1),
    with window coordinates clipped to [0, H] x [0, W] (bilinear sampling of
    the integral image is exact for piecewise-constant images).

That is exactly a separable band matmul with clamped-ramp overlap weights:
    Wx[i, p] = clamp01(p + 1 - (i + x_min)) - clamp01(p + 1 - (i + x_max + 1))
    Wy[j, q] = clamp01(q + 1 - (j + y_min)) - clamp01(q + 1 - (j + y_max + 1))
    out[b, cf] = Wx @ x[b, c] @ Wy^T

The Wx/Wy matrices depend only on the tiny box parameters, so they are built
on the host and shipped to the device; the device kernel is pure TensorE
matmuls in fp16 (fp32 PSUM accumulation), which numpy-validates to ~3e-4
relative error against the fp32 reference.

Sharding: channels across the 8 cores (4 channels/core, all 4 batches), box
parameters replicated per-core as part of each core's W shard.

Step 1 (x side):  V^B[j, f*256+io] = sum_p x[p, j] * Wx[f][io, p]
    lhsT (stationary) = x chunk [p-chunk, j-half], rhs = WxT [p-chunk, 2f*io].
Step 2 (y side):  out[ih*128+io, jo] = sum_j V[j, ...] * Wy[f][jo, j]
    lhsT = V chunk [j-chunk, io-half], rhs = WyT [j-chunk, jo].
"""

import numpy as np

B, C, FN, H, W = 4, 32, 4, 256, 256
N_CORES = 8
C_PER_CORE = C // N_CORES  # 4 channels per core

_PROGRAM_CACHE = {}


def _build_program():
    """Build (once) the SPMD Bass program run identically on all 8 cores."""
    import concourse.bass as bass
    import concourse.tile as tile
    from concourse import bacc, mybir

    nc = bacc.Bacc("TRN2", target_bir_lowering=False, debug=False)
    f16 = mybir.dt.float16
    f32 = mybir.dt.float32

    # Per-core inputs, host-laid-out so every DMA is one contiguous 2D copy:
    # x16[b, c, p, pc*256 + j]          = x[b, c, pc*128 + p, j]
    # wxt[c, p, (fp*2+pc)*512 + fi*256 + io] = Wx[c, 2fp+fi][io, pc*128 + p]
    # wyt[c, j, (f*2+jc)*256 + jo]      = Wy[c, f][jo, jc*128 + j]
    x16 = nc.dram_tensor("x16", [B, C_PER_CORE, 128, 512], f16,
                         kind="ExternalInput").ap()
    wxt = nc.dram_tensor("wxt", [C_PER_CORE, 128, 2048], f16,
                         kind="ExternalInput").ap()
    wyt = nc.dram_tensor("wyt", [C_PER_CORE, 128, 2048], f16,
                         kind="ExternalInput").ap()
    # out_dev[b, c, p, f*512 + a*256 + jo] = out[b, c*FN+f, a*128+p, jo]
    # (host transposes back; keeps store DMAs fully contiguous per partition)
    # fp16 output (|out| <~1e3, fp16 quantization ~5e-4 rel; host upcasts):
    # halves store traffic, and the kernel tail is store-drain bound.
    out = nc.dram_tensor("out", [B, C_PER_CORE, 128, 2048], f16,
                         kind="ExternalOutput").ap()

    with tile.TileContext(nc, pool_alloc_mode="queue") as tc:
        with (
            tc.tile_pool(name="wx", bufs=3) as wx_pool,
            tc.tile_pool(name="wy", bufs=3) as wy_pool,
            tc.tile_pool(name="xin", bufs=10) as x_pool,
            tc.tile_pool(name="v", bufs=8) as v_pool,
            tc.tile_pool(name="osb", bufs=6) as o_pool,
            tc.tile_pool(name="psv", bufs=2, space=bass.MemorySpace.PSUM) as psv_pool,
            tc.tile_pool(name="pso", bufs=4, space=bass.MemorySpace.PSUM) as pso_pool,
        ):
            # Warm the PE clock gate (HAM) during the initial load
            # latency with dependency-free matmuls on scratch data.
            warm_sb = x_pool.tile([128, 128], f16, tag="warm_sb", name="warm_sb")
            nc.vector.memset(warm_sb[:], 0.0)
            warm_ps = pso_pool.tile([128, 512], f32, tag="pso", name="pso")
            for _w in range(32):
                nc.tensor.matmul(warm_ps[:, :128], warm_sb[:], warm_sb[:],
                                 start=True, stop=True)

            xt0 = None
            for c in range(C_PER_CORE):
                # First x tile + first Wx chunk are on the critical path:
                # issue on separate engines/queues, Wx 4-way-split in MM use
                # order so the first matmul (subtile deps) waits only on the
                # first 128KB.  (Per-queue DMA BW is ~110GB/s.)
                if c == 0:
                    xt0 = x_pool.tile([128, 512], f16, tag="x", name="x")
                    nc.gpsimd.dma_start(xt0[:, :256], x16[0, 0][:, :256])
                    nc.gpsimd.dma_start(xt0[:, 256:], x16[0, 0][:, 256:])
                wx_t = wx_pool.tile([128, 2048], f16, tag="wx", name="wx")
                nsplit = 4 if c == 0 else 1
                step = 2048 // nsplit
                for q in range(nsplit):
                    nc.gpsimd.dma_start(wx_t[:, q * step:(q + 1) * step],
                                        wxt[c][:, q * step:(q + 1) * step])
                wy_t = wy_pool.tile([128, 2048], f16, tag="wy", name="wy")
                nsplit = 2 if c == 0 else 1
                step = 2048 // nsplit
                for q in range(nsplit):
                    nc.gpsimd.dma_start(wy_t[:, q * step:(q + 1) * step],
                                        wyt[c][:, q * step:(q + 1) * step])

                for b in range(B):
                    if c == 0 and b == 0:
                        xt = xt0
                    else:
                        xt = x_pool.tile([128, 512], f16, tag="x", name="x")
                        nc.gpsimd.dma_start(xt[:], x16[b, c])

                    # Step 1: psv holds both f-pairs (2 PSUM banks); one
                    # big PSUM->SBUF cast per jh, alternating engine.
                    vt = [v_pool.tile([128, 1024], f16, tag="v", name="v")
                          for _jh in range(2)]
                    for jh in range(2):
                        psv = psv_pool.tile([128, 1024], f32, tag="psv",
                                            name="psv")
                        for fp in range(2):
                            for pc in range(2):
                                nc.tensor.matmul(
                                    psv[:, fp * 512:(fp + 1) * 512],
                                    xt[:, pc * 256 + jh * 128:
                                       pc * 256 + jh * 128 + 128],
                                    wx_t[:, (fp * 2 + pc) * 512:
                                         (fp * 2 + pc) * 512 + 512],
                                    start=(pc == 0),
                                    stop=(pc == 1),
                                )
                        eng = nc.vector.tensor_copy if jh == 0 else nc.scalar.copy
                        eng(vt[jh][:], psv[:])

                    # Step 2
                    osb = o_pool.tile([128, 2048], f16, tag="o", name="osb")
                    for f in range(FN):
                        pso = pso_pool.tile([128, 512], f32, tag="pso",
                                            name="pso")
                        for ih in range(2):
                            for jc in range(2):
                                nc.tensor.matmul(
                                    pso[:, ih * 256:(ih + 1) * 256],
                                    vt[jc][:, f * 256 + ih * 128:
                                           f * 256 + ih * 128 + 128],
                                    wy_t[:, (f * 2 + jc) * 256:
                                         (f * 2 + jc) * 256 + 256],
                                    start=(jc == 0),
                                    stop=(jc == 1),
                                )
                        dst = osb[:, f * 512:(f + 1) * 512]
                        eng = nc.vector.tensor_copy if f % 2 == 0 else nc.scalar.copy
                        eng(dst[:], pso[:])
                        if c == C_PER_CORE - 1 and b == B - 1:
                            # final tile: store each f-chunk as soon as it is
                            # copied, shortening the kernel tail
                            nc.sync.dma_start(
                                out[b, c][:, f * 512:(f + 1) * 512], dst[:])
                    if not (c == C_PER_CORE - 1 and b == B - 1):
                        # contiguous store on sync, split across DMA queues
                        # (4-way near the end to shorten the final drain)
                        nsp = 4 if (c == C_PER_CORE - 1 and b == B - 2) else 2
                        stp = 2048 // nsp
                        for q in range(nsp):
                            nc.sync.dma_start(
                                out[b, c][:, q * stp:(q + 1) * stp],
                                osb[:, q * stp:(q + 1) * stp])

    nc.compile()
    return nc


def _get_program():
    if "nc" not in _PROGRAM_CACHE:
        _PROGRAM_CACHE["nc"] = _build_program()
    return _PROGRAM_CACHE["nc"]


def _band(mn, mx, dim):
    """Overlap weights W[i, p] of clipped window [i+mn, i+mx+1) with cell
    [p, p+1), built in fp64."""
    i = np.arange(dim, dtype=np.float64)[:, None]
    p = np.arange(dim, dtype=np.float64)[None, :]
    lo = i + float(mn)
    hi = i + float(mx) + 1.0
    return np.clip(p + 1.0 - lo, 0.0, 1.0) - np.clip(p + 1.0 - hi, 0.0, 1.0)


def _prepare_in_maps(input, x_min, x_max, y_min, y_max):
    # x16[b, c, p, pc*256 + j] = x[b, c, pc*128 + p, j]
    x16_full = np.ascontiguousarray(
        input.astype(np.float16).reshape(B, C, 2, 128, 256)
        .transpose(0, 1, 3, 2, 4).reshape(B, C, 128, 512))

    in_maps = []
    for core in range(N_CORES):
        c0 = core * C_PER_CORE
        wxt = np.empty((C_PER_CORE, 128, 2048), dtype=np.float16)
        wyt = np.empty((C_PER_CORE, 128, 2048), dtype=np.float16)
        for cl in range(C_PER_CORE):
            c = c0 + cl
            for f in range(FN):
                WxT = _band(x_min[c, f], x_max[c, f], H).T.astype(np.float16)
                WyT = _band(y_min[c, f], y_max[c, f], W).T.astype(np.float16)
                fp, fi = f // 2, f % 2
                for pc in range(2):
                    base = (fp * 2 + pc) * 512 + fi * 256
                    wxt[cl, :, base:base + 256] = WxT[pc * 128:(pc + 1) * 128]
                for jc in range(2):
                    base = (f * 2 + jc) * 256
                    wyt[cl, :, base:base + 256] = WyT[jc * 128:(jc + 1) * 128]
        in_maps.append({
            "x16": np.ascontiguousarray(x16_full[:, c0:c0 + C_PER_CORE]),
            "wxt": wxt,
            "wyt": wyt,
        })
    return in_maps


def run(input, x_min, x_max, y_min, y_max, trace=False):
    """Run the SPMD kernel; returns (full_output, BassKernelResults)."""
    from concourse.bass_utils import run_bass_kernel_spmd

    nc = _get_program()
    in_maps = _prepare_in_maps(
        np.asarray(input, dtype=np.float32),
        np.asarray(x_min, dtype=np.float64),
        np.asarray(x_max, dtype=np.float64),
        np.asarray(y_min, dtype=np.float64),
        np.asarray(y_max, dtype=np.float64),
    )
    res = run_bass_kernel_spmd(nc, in_maps, list(range(N_CORES)), trace=trace)
    # out_dev[b, c, p, f*512 + a*256 + jo] -> out[b, c*FN+f, a*128+p, jo]
    parts = []
    for i in range(N_CORES):
        o = res.results[i]["out"].astype(np.float32).reshape(
            B, C_PER_CORE, 128, FN, 2, 256)
        parts.append(o.transpose(0, 1, 3, 4, 2, 5).reshape(
            B, C_PER_CORE * FN, 256, 256))
    full = np.ascontiguousarray(np.concatenate(parts, axis=1))
    return full, res


def kernel(input, x_min, x_max, y_min, y_max):
    full, _ = run(input, x_min, x_max, y_min, y_max)
    return full



# revision 4
# speedup vs baseline: 1.0419x; 1.0419x over previous
"""BoxConv2d Trainium2 kernel (band-sparse v2).

out[b, c*FN+f] = Wx[c,f] @ x[b,c] @ Wy[c,f]^T with clamped-ramp band matrices
(see kernel_v1_backup.py for the derivation).  This version exploits the BAND
structure of Wx/Wy: for a box of height h, the 256x256 band matrix has one
contiguous run of nonzero columns per 128-row chunk, so

  stage 1 (x side, psum V[j, f*256+io], contraction p in 2 chunks):
     pc0 covers io in [0, W0), pc1 covers io in [L1, 256)  (W0+W1>=256)
     -> per (jh, f-pair-bank): 3 matmuls  T(pc0,[0,W0)) B(pc1,[L1,W0))
        C(pc1,[W0,256)) with 2-block column APs (the pair packed side by side)
  stage 2 (y side, psum out[io, jo], contraction j in 2 chunks):
     per (f, ih): 3 matmuls  T(jc0,[0,G0))  B(jc1,[M1,G0))  C(jc1,[G0,256))

start/stop flags are chosen so every psum element's first writer has
start=True (T/C) and accumulating writers (B) land on freshly set
has_written bits, safe under both bank-clear and per-element semantics.

Stage 2 is LDWEIGHTS-bound (~97ns per 128-col stationary, 16 per (b,c)), so
stage 2 of (b,c) t is interleaved with stage 1 of t+2 to keep the PE array fed.

The matmul column ranges are baked into the (single, SPMD) program, so ranges
are unified as max-over-cores; channels are assigned to core slots sorted by
band width, and filters permuted per channel, to keep the unified ranges tight.
Weights ship band-packed (only the nonzero column runs), ~2.6MB/core vs 4MB.
"""

import os
import numpy as np

B, C, FN, H, W = 4, 32, 4, 256, 256
N_CORES = 8
C_PER_CORE = C // N_CORES  # 4 channel slots per core

_PROGRAM_CACHE = {}

WARM_MMS = int(os.environ.get("BOXC_WARM", "22"))
LAG = int(os.environ.get("BOXC_LAG", "2"))  # stage2 interleave lag in (b,c) units


def _band(mn, mx, dim):
    """Overlap weights W[i, p] of clipped window [i+mn, i+mx+1) with cell
    [p, p+1), fp64."""
    i = np.arange(dim, dtype=np.float64)[:, None]
    p = np.arange(dim, dtype=np.float64)[None, :]
    lo = i + float(mn)
    hi = i + float(mx) + 1.0
    return np.clip(p + 1.0 - lo, 0.0, 1.0) - np.clip(p + 1.0 - hi, 0.0, 1.0)


def _pad8up(v):
    return int(min(256, (int(v) + 7) & ~7))


def _pad8dn(v):
    return int(max(0, int(v) & ~7))


def _support_cols(block):
    """[lo, hi) column support of a [128, 256] block (hi=0 if empty)."""
    nz = np.flatnonzero(np.abs(block).max(axis=0) > 0)
    if len(nz) == 0:
        return 0, 0
    return int(nz[0]), int(nz[-1]) + 1


def _plan(x_min, x_max, y_min, y_max):
    """Channel/filter assignment + unified column ranges + packed weights."""
    # Per (channel, filter) band matrices, transposed ([p, io] layout).
    WxT = np.empty((C, FN, 256, 256), np.float16)
    WyT = np.empty((C, FN, 256, 256), np.float16)
    h0 = np.zeros((C, FN), np.int32)   # x side: pc0 support is [0, h0)
    l1 = np.zeros((C, FN), np.int32)   # x side: pc1 support is [l1, 256)
    g0 = np.zeros((C, FN), np.int32)   # y side: jc0 support [0, g0)
    m1 = np.zeros((C, FN), np.int32)   # y side: jc1 support [m1, 256)
    for c in range(C):
        for f in range(FN):
            wx = _band(x_min[c, f], x_max[c, f], H).T
            wy = _band(y_min[c, f], y_max[c, f], W).T
            WxT[c, f] = wx.astype(np.float16)
            WyT[c, f] = wy.astype(np.float16)
            _, h = _support_cols(wx[0:128])
            lo, _ = _support_cols(wx[128:256])
            h0[c, f], l1[c, f] = max(h, 8), min(lo, 248)
            _, h = _support_cols(wy[0:128])
            lo, _ = _support_cols(wy[128:256])
            g0[c, f], m1[c, f] = max(h, 8), min(lo, 248)

    # Filter permutation per channel: sort by x-extent desc so pair (0,1) is
    # the wide pair -> pair-max padding is tight.
    xext = h0 + (256 - l1)
    perm = np.argsort(-xext, axis=1, kind="stable")  # [C, FN]
    # Channel order: sort desc by total extent; slot cl takes ranks
    # [8cl, 8cl+8), one per core -> channels in a slot have similar widths.
    yext = g0 + (256 - m1)
    key = (xext + yext).sum(axis=1)
    order = np.argsort(-key, kind="stable")
    assign = [[int(order[8 * cl + k]) for cl in range(C_PER_CORE)]
              for k in range(N_CORES)]

    # Unified (max-over-cores) ranges per (slot, position).
    W0u = np.zeros((C_PER_CORE, 2), np.int32)
    L1u = np.zeros((C_PER_CORE, 2), np.int32)
    G0u = np.zeros((C_PER_CORE, FN), np.int32)
    M1u = np.zeros((C_PER_CORE, FN), np.int32)
    for cl in range(C_PER_CORE):
        chans = [assign[k][cl] for k in range(N_CORES)]
        for bank in range(2):
            hs, ls = [], []
            for ch in chans:
                for fp in (2 * bank, 2 * bank + 1):
                    f = perm[ch][fp]
                    hs.append(h0[ch, f])
                    ls.append(l1[ch, f])
            W0u[cl, bank] = _pad8up(max(hs))
            L1u[cl, bank] = _pad8dn(min(ls))
        for fp in range(FN):
            gs, ms = [], []
            for ch in chans:
                f = perm[ch][fp]
                gs.append(g0[ch, f])
                ms.append(m1[ch, f])
            G0u[cl, fp] = _pad8up(max(gs))
            M1u[cl, fp] = _pad8dn(min(ms))

    # Weight layouts.  Per slot cl the x-weight columns are
    #   [bank0 A | bank1 A | bank0 B | bank1 B]  (A = pc0 pair 2*W0u cols,
    #   B = pc1 pair 2*W1u cols, W1u = 256 - L1u), A-blocks first so the
    #   startup load covers stage-1 pc0 of slot 0 quickly.
    xoffA = np.zeros((C_PER_CORE, 2), np.int64)
    xoffB = np.zeros((C_PER_CORE, 2), np.int64)
    xcl0 = np.zeros(C_PER_CORE + 1, np.int64)  # slot col start
    pos = 0
    for cl in range(C_PER_CORE):
        xcl0[cl] = pos
        for bank in range(2):
            xoffA[cl, bank] = pos
            pos += 2 * int(W0u[cl, bank])
        for bank in range(2):
            xoffB[cl, bank] = pos
            pos += 2 * (256 - int(L1u[cl, bank]))
    xcl0[C_PER_CORE] = pos
    XW = int(pos)

    yoff = np.zeros((C_PER_CORE, FN), np.int64)  # jc0 block start per (cl, f)
    ycl0 = np.zeros(C_PER_CORE + 1, np.int64)
    pos = 0
    for cl in range(C_PER_CORE):
        ycl0[cl] = pos
        for fp in range(FN):
            yoff[cl, fp] = pos
            pos += int(G0u[cl, fp]) + (256 - int(M1u[cl, fp]))
    ycl0[C_PER_CORE] = pos
    YW = int(pos)

    return dict(WxT=WxT, WyT=WyT, perm=perm, assign=assign,
                W0u=W0u, L1u=L1u, G0u=G0u, M1u=M1u,
                xoffA=xoffA, xoffB=xoffB, xcl0=xcl0, XW=XW,
                yoff=yoff, ycl0=ycl0, YW=YW)


def _range_key(plan):
    return (tuple(plan["W0u"].ravel()), tuple(plan["L1u"].ravel()),
            tuple(plan["G0u"].ravel()), tuple(plan["M1u"].ravel()),
            WARM_MMS, LAG)


def _build_program(plan):
    import concourse.bass as bass
    import concourse.tile as tile
    from concourse import bacc, mybir

    W0u, L1u = plan["W0u"], plan["L1u"]
    G0u, M1u = plan["G0u"], plan["M1u"]
    xoffA, xoffB = plan["xoffA"], plan["xoffB"]
    yoff = plan["yoff"]
    xcl0, ycl0 = plan["xcl0"], plan["ycl0"]
    XW, YW = plan["XW"], plan["YW"]

    nc = bacc.Bacc("TRN2", target_bir_lowering=False, debug=False)
    f16 = mybir.dt.float16
    f32 = mybir.dt.float32

    # xc[cl][p, b*512 + pc*256 + j] = x[b, ch(cl), pc*128+p, j]
    xcd = nc.dram_tensor("xc", [C_PER_CORE, 128, 2048], f16,
                         kind="ExternalInput").ap()
    wxd = nc.dram_tensor("wx", [128, XW], f16, kind="ExternalInput").ap()
    wyd = nc.dram_tensor("wy", [128, YW], f16, kind="ExternalInput").ap()
    # out[b, cl, p, fpos*512 + ih*256 + jo] = out[b, ch, ih*128+p, jo]
    out = nc.dram_tensor("out", [B, C_PER_CORE, 128, 2048], f16,
                         kind="ExternalOutput").ap()

    NT = B * C_PER_CORE  # 16 (b,c) units; t = cl*4 + b

    with tile.TileContext(nc, pool_alloc_mode="queue") as tc:
        with (
            tc.tile_pool(name="xc", bufs=2) as xc_pool,
            tc.tile_pool(name="wx", bufs=2) as wx_pool,
            tc.tile_pool(name="wy", bufs=2) as wy_pool,
            tc.tile_pool(name="vt", bufs=6) as vt_pool,
            tc.tile_pool(name="osb", bufs=3) as o_pool,
            tc.tile_pool(name="warm", bufs=1) as warm_pool,
            tc.tile_pool(name="psv", bufs=2, space=bass.MemorySpace.PSUM) as psv_pool,
            tc.tile_pool(name="pso", bufs=4, space=bass.MemorySpace.PSUM) as pso_pool,
        ):
            # ---- loads ----------------------------------------------------
            # First chunks on the two HWDGE queues (sync + scalar) for low
            # first-byte latency; the bulk on gpsimd (SWDGE).
            xc_t = [None] * C_PER_CORE
            wx_t = [None] * C_PER_CORE
            wy_t = [None] * C_PER_CORE
            for cl in range(C_PER_CORE):
                xc_t[cl] = xc_pool.tile([128, 2048], f16, tag="xc", name="xc")
                wx_t[cl] = wx_pool.tile([128, int(xcl0[cl + 1] - xcl0[cl])],
                                        f16, tag="wx", name="wx")
                wy_t[cl] = wy_pool.tile([128, int(ycl0[cl + 1] - ycl0[cl])],
                                        f16, tag="wy", name="wy")
            aw0 = int(xoffB[0, 0] - xcl0[0])  # A-blocks of slot 0
            nc.sync.dma_start(wx_t[0][:, :aw0], wxd[:, :aw0])
            nc.scalar.dma_start(xc_t[0][:, :512], xcd[0][:, :512])
            nc.sync.dma_start(wx_t[0][:, aw0:], wxd[:, aw0:int(xcl0[1])])
            nc.scalar.dma_start(xc_t[0][:, 512:], xcd[0][:, 512:])
            nc.gpsimd.dma_start(wy_t[0][:], wyd[:, :int(ycl0[1])])
            for cl in range(1, C_PER_CORE):
                nc.gpsimd.dma_start(wx_t[cl][:],
                                    wxd[:, int(xcl0[cl]):int(xcl0[cl + 1])])
                nc.gpsimd.dma_start(xc_t[cl][:], xcd[cl])
                nc.gpsimd.dma_start(wy_t[cl][:],
                                    wyd[:, int(ycl0[cl]):int(ycl0[cl + 1])])

            # ---- warmup ---------------------------------------------------
            warm_sb = warm_pool.tile([128, 128], f16, tag="warm", name="warm")
            nc.vector.memset(warm_sb[:], 0.0)
            warm_ps = pso_pool.tile([128, 512], f32, tag="pso", name="pso")
            for _ in range(WARM_MMS):
                nc.tensor.matmul(warm_ps[:, :128], warm_sb[:], warm_sb[:],
                                 start=True, stop=True)

            # ---- per-(b,c) emission --------------------------------------
            psv_tiles = {}   # t -> [psv_jh0, psv_jh1]
            vt_tiles = {}    # t -> [vt0, vt1]
            osb_tiles = {}
            pso_tiles = {}   # (t, fp) -> tile

            def s1_unit(t, jh, pc):
                """Stage-1 matmuls for one (jh, pc)."""
                cl, b = divmod(t, 4)
                if jh == 0 and pc == 0:
                    psv_tiles[t] = [
                        psv_pool.tile([128, 1024], f32, tag="psv", name="psv")
                        for _ in range(2)]
                psv = psv_tiles[t][jh]
                xt = xc_t[cl][:, b * 512 + pc * 256 + jh * 128:
                              b * 512 + pc * 256 + jh * 128 + 128]
                for bank in range(2):
                    W0 = int(W0u[cl, bank])
                    L1 = int(L1u[cl, bank])
                    W1 = 256 - L1
                    pblk = psv[:, bank * 512:(bank + 1) * 512].rearrange(
                        "p (g c) -> p g c", g=2)
                    if pc == 0:
                        rhs = wx_t[cl][:, int(xoffA[cl, bank] - xcl0[cl]):
                                       int(xoffA[cl, bank] - xcl0[cl]) + 2 * W0]
                        nc.tensor.matmul(pblk[:, :, 0:W0], xt, rhs,
                                         start=True, stop=False)
                    else:
                        boff = int(xoffB[cl, bank] - xcl0[cl])
                        wblk = wx_t[cl][:, boff:boff + 2 * W1].rearrange(
                            "p (g w) -> p g w", g=2)
                        nc.tensor.matmul(pblk[:, :, L1:W0], xt,
                                         wblk[:, :, 0:W0 - L1],
                                         start=False, stop=True)
                        if W0 < 256:
                            nc.tensor.matmul(pblk[:, :, W0:256], xt,
                                             wblk[:, :, W0 - L1:W0 - L1 + 256 - W0],
                                             start=True, stop=True)

            def s1_copy(t, jh):
                if t not in vt_tiles:
                    vt_tiles[t] = [
                        vt_pool.tile([128, 1024], f16, tag="vt", name="vt")
                        for _ in range(2)]
                eng = nc.vector.tensor_copy if jh == 0 else nc.scalar.copy
                eng(vt_tiles[t][jh][:], psv_tiles[t][jh][:])

            def s2_unit(t, fp, ih):
                """Stage-2 matmuls for one (f-position, ih)."""
                cl, b = divmod(t, 4)
                if ih == 0:
                    pso_tiles[(t, fp)] = pso_pool.tile([128, 512], f32,
                                                       tag="pso", name="pso")
                pso = pso_tiles[(t, fp)]
                G0 = int(G0u[cl, fp])
                M1 = int(M1u[cl, fp])
                yo0 = int(yoff[cl, fp] - ycl0[cl])
                yo1 = yo0 + G0
                vt0, vt1 = vt_tiles[t]
                st0 = vt0[:, fp * 256 + ih * 128: fp * 256 + ih * 128 + 128]
                st1 = vt1[:, fp * 256 + ih * 128: fp * 256 + ih * 128 + 128]
                o = ih * 256
                nc.tensor.matmul(pso[:, o:o + G0], st0,
                                 wy_t[cl][:, yo0:yo0 + G0],
                                 start=True, stop=False)
                nc.tensor.matmul(pso[:, o + M1:o + G0], st1,
                                 wy_t[cl][:, yo1:yo1 + G0 - M1],
                                 start=False, stop=True)
                if G0 < 256:
                    nc.tensor.matmul(pso[:, o + G0:o + 256], st1,
                                     wy_t[cl][:, yo1 + G0 - M1:yo1 + 256 - M1],
                                     start=True, stop=True)

            def s2_copy(t, fp, eng_idx):
                cl, b = divmod(t, 4)
                if t not in osb_tiles:
                    osb_tiles[t] = o_pool.tile([128, 2048], f16, tag="o",
                                               name="osb")
                dst = osb_tiles[t][:, fp * 512:(fp + 1) * 512]
                eng = nc.vector.tensor_copy if eng_idx == 0 else nc.scalar.copy
                eng(dst[:], pso_tiles[(t, fp)][:])

            def store(t, split):
                cl, b = divmod(t, 4)
                osb = osb_tiles[t]
                if split == 1:
                    nc.sync.dma_start(out[b, cl], osb[:])
                else:
                    stp = 2048 // split
                    for q in range(split):
                        nc.sync.dma_start(out[b, cl][:, q * stp:(q + 1) * stp],
                                          osb[:, q * stp:(q + 1) * stp])

            def emit_s1(t):
                """Full stage-1 of t as a unit list (callables)."""
                units = []
                for jh in range(2):
                    units.append(lambda t=t, jh=jh: s1_unit(t, jh, 0))
                    def u(t=t, jh=jh):
                        s1_unit(t, jh, 1)
                        s1_copy(t, jh)
                    units.append(u)
                return units

            def emit_s2(t, last=False):
                units = []
                # copy engines: f0,f1,f2 on vector, f3 on scalar (balance);
                # on the final unit everything critical goes to vector and
                # each f stores as soon as it is copied (short tail).
                engs = [0, 0, 1, 0] if last else [0, 0, 0, 1]
                for fp in range(FN):
                    units.append(lambda t=t, fp=fp: s2_unit(t, fp, 0))
                    def u(t=t, fp=fp):
                        s2_unit(t, fp, 1)
                        s2_copy(t, fp, engs[fp])
                        if last:
                            cl, b = divmod(t, 4)
                            nc.sync.dma_start(
                                out[b, cl][:, fp * 512:(fp + 1) * 512],
                                osb_tiles[t][:, fp * 512:(fp + 1) * 512])
                        elif fp == 3:
                            store(t, 1)
                    units.append(u)
                return units

            # interleave: window t runs S1(t) units + S2(t-LAG) units
            for t in range(NT + LAG):
                s1u = emit_s1(t) if t < NT else []
                t2 = t - LAG
                s2u = emit_s2(t2, last=(t2 == NT - 1)) if t2 >= 0 else []
                if not s2u:
                    for u in s1u:
                        u()
                else:
                    n1, n2 = len(s1u), len(s2u)
                    k2 = 0
                    for k1 in range(n1):
                        s1u[k1]()
                        take = ((k1 + 1) * n2) // max(n1, 1) - k2
                        for _ in range(take):
                            s2u[k2]()
                            k2 += 1
                    while k2 < n2:
                        s2u[k2]()
                        k2 += 1

    nc.compile()
    return nc


def _get_program(plan):
    key = _range_key(plan)
    if key not in _PROGRAM_CACHE:
        _PROGRAM_CACHE[key] = _build_program(plan)
    return _PROGRAM_CACHE[key]


def _prepare_in_maps(x, plan):
    x16 = x.astype(np.float16)
    WxT, WyT = plan["WxT"], plan["WyT"]
    perm, assign = plan["perm"], plan["assign"]
    W0u, L1u, G0u, M1u = plan["W0u"], plan["L1u"], plan["G0u"], plan["M1u"]
    xoffA, xoffB, yoff = plan["xoffA"], plan["xoffB"], plan["yoff"]
    XW, YW = plan["XW"], plan["YW"]

    in_maps = []
    for k in range(N_CORES):
        xc = np.zeros((C_PER_CORE, 128, 2048), np.float16)
        wx = np.zeros((128, XW), np.float16)
        wy = np.zeros((128, YW), np.float16)
        for cl in range(C_PER_CORE):
            ch = assign[k][cl]
            # xc[cl][p, b*512 + pc*256 + j]
            xc[cl] = (x16[:, ch].reshape(B, 2, 128, 256)
                      .transpose(2, 0, 1, 3).reshape(128, 2048))
            for bank in range(2):
                W0 = int(W0u[cl, bank])
                L1 = int(L1u[cl, bank])
                W1 = 256 - L1
                for i, fp in enumerate((2 * bank, 2 * bank + 1)):
                    f = int(perm[ch][fp])
                    a = int(xoffA[cl, bank]) + i * W0
                    wx[:, a:a + W0] = WxT[ch, f][0:128, 0:W0]
                    bo = int(xoffB[cl, bank]) + i * W1
                    wx[:, bo:bo + W1] = WxT[ch, f][128:256, L1:256]
            for fp in range(FN):
                f = int(perm[ch][fp])
                G0 = int(G0u[cl, fp])
                M1 = int(M1u[cl, fp])
                a = int(yoff[cl, fp])
                wy[:, a:a + G0] = WyT[ch, f][0:128, 0:G0]
                wy[:, a + G0:a + G0 + 256 - M1] = WyT[ch, f][128:256, M1:256]
        in_maps.append({"xc": xc, "wx": wx, "wy": wy})
    return in_maps


def run(input, x_min, x_max, y_min, y_max, trace=False):
    from concourse.bass_utils import run_bass_kernel_spmd

    x = np.asarray(input, dtype=np.float32)
    plan = _plan(np.asarray(x_min, np.float64), np.asarray(x_max, np.float64),
                 np.asarray(y_min, np.float64), np.asarray(y_max, np.float64))
    nc = _get_program(plan)
    in_maps = _prepare_in_maps(x, plan)
    res = run_bass_kernel_spmd(nc, in_maps, list(range(N_CORES)), trace=trace)

    perm, assign = plan["perm"], plan["assign"]
    full = np.empty((B, C * FN, 256, 256), np.float32)
    for k in range(N_CORES):
        o = res.results[k]["out"].astype(np.float32)
        # o[b, cl, p, fp*512 + ih*256 + jo]
        o = o.reshape(B, C_PER_CORE, 128, FN, 2, 256)
        o = o.transpose(0, 1, 3, 4, 2, 5)  # [b, cl, fp, ih, p, jo]
        for cl in range(C_PER_CORE):
            ch = assign[k][cl]
            idx = ch * FN + perm[ch]  # output channel per f-position
            full[:, idx] = o[:, cl].reshape(B, FN, 256, 256)
    return full, res


def kernel(input, x_min, x_max, y_min, y_max):
    full, _ = run(input, x_min, x_max, y_min, y_max)
    return full


# revision 10
# speedup vs baseline: 1.1694x; 1.1224x over previous
"""BoxConv2d Trainium2 kernel (band-sparse v2).

out[b, c*FN+f] = Wx[c,f] @ x[b,c] @ Wy[c,f]^T with clamped-ramp band matrices
(see kernel_v1_backup.py for the derivation).  This version exploits the BAND
structure of Wx/Wy: for a box of height h, the 256x256 band matrix has one
contiguous run of nonzero columns per 128-row chunk, so

  stage 1 (x side, psum V[j, f*256+io], contraction p in 2 chunks):
     pc0 covers io in [0, W0), pc1 covers io in [L1, 256)  (W0+W1>=256)
     -> per (jh, f-pair-bank): 3 matmuls  T(pc0,[0,W0)) B(pc1,[L1,W0))
        C(pc1,[W0,256)) with 2-block column APs (the pair packed side by side)
  stage 2 (y side, psum out[io, jo], contraction j in 2 chunks):
     per (f, ih): 3 matmuls  T(jc0,[0,G0))  B(jc1,[M1,G0))  C(jc1,[G0,256))

start/stop flags are chosen so every psum element's first writer has
start=True (T/C) and accumulating writers (B) land on freshly set
has_written bits, safe under both bank-clear and per-element semantics.

Stage 2 is LDWEIGHTS-bound (~97ns per 128-col stationary, 16 per (b,c)), so
stage 2 of (b,c) t is interleaved with stage 1 of t+2 to keep the PE array fed.

The matmul column ranges are baked into the (single, SPMD) program, so ranges
are unified as max-over-cores; channels are assigned to core slots sorted by
band width, and filters permuted per channel, to keep the unified ranges tight.
Weights ship band-packed (only the nonzero column runs), ~2.6MB/core vs 4MB.
"""

import os
import numpy as np

B, C, FN, H, W = 4, 32, 4, 256, 256
N_CORES = 8
C_PER_CORE = C // N_CORES  # 4 channel slots per core

_PROGRAM_CACHE = {}

WARM_MMS = int(os.environ.get("BOXC_WARM", "14"))
LAG = int(os.environ.get("BOXC_LAG", "2"))  # stage2 interleave lag in (b,c) units


def _band(mn, mx, dim):
    """Overlap weights W[i, p] of clipped window [i+mn, i+mx+1) with cell
    [p, p+1), fp64."""
    i = np.arange(dim, dtype=np.float64)[:, None]
    p = np.arange(dim, dtype=np.float64)[None, :]
    lo = i + float(mn)
    hi = i + float(mx) + 1.0
    return np.clip(p + 1.0 - lo, 0.0, 1.0) - np.clip(p + 1.0 - hi, 0.0, 1.0)


def _pad8up(v):
    return int(min(256, (int(v) + 7) & ~7))


def _pad8dn(v):
    return int(max(0, int(v) & ~7))


def _support_cols(block):
    """[lo, hi) column support of a [128, 256] block (hi=0 if empty)."""
    nz = np.flatnonzero(np.abs(block).max(axis=0) > 0)
    if len(nz) == 0:
        return 0, 0
    return int(nz[0]), int(nz[-1]) + 1


def _plan(x_min, x_max, y_min, y_max):
    """Channel/filter assignment + unified column ranges + packed weights."""
    # Per (channel, filter) band matrices, transposed ([p, io] layout).
    WxT = np.empty((C, FN, 256, 256), np.float16)
    WyT = np.empty((C, FN, 256, 256), np.float16)
    h0 = np.zeros((C, FN), np.int32)   # x side: pc0 support is [0, h0)
    l1 = np.zeros((C, FN), np.int32)   # x side: pc1 support is [l1, 256)
    g0 = np.zeros((C, FN), np.int32)   # y side: jc0 support [0, g0)
    m1 = np.zeros((C, FN), np.int32)   # y side: jc1 support [m1, 256)
    for c in range(C):
        for f in range(FN):
            wx = _band(x_min[c, f], x_max[c, f], H).T
            wy = _band(y_min[c, f], y_max[c, f], W).T
            WxT[c, f] = wx.astype(np.float16)
            WyT[c, f] = wy.astype(np.float16)
            _, h = _support_cols(wx[0:128])
            lo, _ = _support_cols(wx[128:256])
            h0[c, f], l1[c, f] = max(h, 8), min(lo, 248)
            _, h = _support_cols(wy[0:128])
            lo, _ = _support_cols(wy[128:256])
            g0[c, f], m1[c, f] = max(h, 8), min(lo, 248)

    # Filter permutation per channel: sort by x-extent desc so pair (0,1) is
    # the wide pair -> pair-max padding is tight.
    xext = h0 + (256 - l1)
    perm = np.argsort(-xext, axis=1, kind="stable")  # [C, FN]
    # Channel order: sort desc by total extent; slot cl takes ranks
    # [8cl, 8cl+8), one per core -> channels in a slot have similar widths.
    yext = g0 + (256 - m1)
    key = (xext + yext).sum(axis=1)
    order = np.argsort(-key, kind="stable")
    assign = [[int(order[8 * cl + k]) for cl in range(C_PER_CORE)]
              for k in range(N_CORES)]

    # Unified (max-over-cores) ranges per (slot, position).
    W0u = np.zeros((C_PER_CORE, 2), np.int32)
    L1u = np.zeros((C_PER_CORE, 2), np.int32)
    G0u = np.zeros((C_PER_CORE, FN), np.int32)
    M1u = np.zeros((C_PER_CORE, FN), np.int32)
    for cl in range(C_PER_CORE):
        chans = [assign[k][cl] for k in range(N_CORES)]
        for bank in range(2):
            hs, ls = [], []
            for ch in chans:
                for fp in (2 * bank, 2 * bank + 1):
                    f = perm[ch][fp]
                    hs.append(h0[ch, f])
                    ls.append(l1[ch, f])
            W0u[cl, bank] = _pad8up(max(hs))
            L1u[cl, bank] = _pad8dn(min(ls))
        for fp in range(FN):
            gs, ms = [], []
            for ch in chans:
                f = perm[ch][fp]
                gs.append(g0[ch, f])
                ms.append(m1[ch, f])
            G0u[cl, fp] = _pad8up(max(gs))
            M1u[cl, fp] = _pad8dn(min(ms))

    # Weight layouts.  Per slot cl the x-weight columns are
    #   [bank0 A | bank1 A | bank0 B | bank1 B]  (A = pc0 pair 2*W0u cols,
    #   B = pc1 pair 2*W1u cols, W1u = 256 - L1u), A-blocks first so the
    #   startup load covers stage-1 pc0 of slot 0 quickly.
    xoffA = np.zeros((C_PER_CORE, 2), np.int64)
    xoffB = np.zeros((C_PER_CORE, 2), np.int64)
    xcl0 = np.zeros(C_PER_CORE + 1, np.int64)  # slot col start
    pos = 0
    for cl in range(C_PER_CORE):
        xcl0[cl] = pos
        for bank in range(2):
            xoffA[cl, bank] = pos
            pos += 2 * int(W0u[cl, bank])
        for bank in range(2):
            xoffB[cl, bank] = pos
            pos += 2 * (256 - int(L1u[cl, bank]))
    xcl0[C_PER_CORE] = pos
    XW = int(pos)

    yoff = np.zeros((C_PER_CORE, FN), np.int64)  # jc0 block start per (cl, f)
    ycl0 = np.zeros(C_PER_CORE + 1, np.int64)
    pos = 0
    for cl in range(C_PER_CORE):
        ycl0[cl] = pos
        for fp in range(FN):
            yoff[cl, fp] = pos
            pos += int(G0u[cl, fp]) + (256 - int(M1u[cl, fp]))
    ycl0[C_PER_CORE] = pos
    YW = int(pos)

    return dict(WxT=WxT, WyT=WyT, perm=perm, assign=assign,
                W0u=W0u, L1u=L1u, G0u=G0u, M1u=M1u,
                xoffA=xoffA, xoffB=xoffB, xcl0=xcl0, XW=XW,
                yoff=yoff, ycl0=ycl0, YW=YW)


def _range_key(plan):
    return (tuple(plan["W0u"].ravel()), tuple(plan["L1u"].ravel()),
            tuple(plan["G0u"].ravel()), tuple(plan["M1u"].ravel()),
            WARM_MMS, LAG)


def _build_program(plan):
    import concourse.bass as bass
    import concourse.tile as tile
    from concourse import bacc, mybir

    W0u, L1u = plan["W0u"], plan["L1u"]
    G0u, M1u = plan["G0u"], plan["M1u"]
    xoffA, xoffB = plan["xoffA"], plan["xoffB"]
    yoff = plan["yoff"]
    xcl0, ycl0 = plan["xcl0"], plan["ycl0"]
    XW, YW = plan["XW"], plan["YW"]

    nc = bacc.Bacc("TRN2", target_bir_lowering=False, debug=False)
    f16 = mybir.dt.float16
    f32 = mybir.dt.float32

    # xc[cl][p, b*512 + pc*256 + j] = x[b, ch(cl), pc*128+p, j]
    xcd = nc.dram_tensor("xc", [C_PER_CORE, 128, 2048], f16,
                         kind="ExternalInput").ap()
    wxd = nc.dram_tensor("wx", [128, XW], f16, kind="ExternalInput").ap()
    wyd = nc.dram_tensor("wy", [128, YW], f16, kind="ExternalInput").ap()
    # out[b, cl, p, fpos*512 + ih*256 + jo] = out[b, ch, ih*128+p, jo]
    out = nc.dram_tensor("out", [B, C_PER_CORE, 128, 2048], f16,
                         kind="ExternalOutput").ap()

    NT = B * C_PER_CORE  # 16 (b,c) units; t = cl*4 + b

    with tile.TileContext(nc, pool_alloc_mode="queue") as tc:
        with (
            tc.tile_pool(name="xc", bufs=2) as xc_pool,
            tc.tile_pool(name="wx", bufs=2) as wx_pool,
            tc.tile_pool(name="wy", bufs=2) as wy_pool,
            tc.tile_pool(name="vt", bufs=6) as vt_pool,
            tc.tile_pool(name="osb", bufs=3) as o_pool,
            tc.tile_pool(name="warm", bufs=1) as warm_pool,
            tc.tile_pool(name="psv", bufs=2, space=bass.MemorySpace.PSUM) as psv_pool,
            tc.tile_pool(name="pso", bufs=2, space=bass.MemorySpace.PSUM) as pso_pool,
        ):
            # ---- warm stationary first: gpsimd queue is otherwise busy ----
            warm_sb = warm_pool.tile([128, 128], f16, tag="warm", name="warm")
            nc.gpsimd.memset(warm_sb[:], 0.0)

            # ---- loads ----------------------------------------------------
            # First chunks on the two HWDGE queues (sync + scalar) for low
            # first-byte latency; the bulk on gpsimd (SWDGE).
            xc_t = [None] * C_PER_CORE
            wx_t = [None] * C_PER_CORE
            wy_t = [None] * C_PER_CORE
            for cl in range(C_PER_CORE):
                xc_t[cl] = xc_pool.tile([128, 2048], f16, tag="xc", name="xc")
                wx_t[cl] = wx_pool.tile([128, int(xcl0[cl + 1] - xcl0[cl])],
                                        f16, tag="wx", name="wx")
                wy_t[cl] = wy_pool.tile([128, int(ycl0[cl + 1] - ycl0[cl])],
                                        f16, tag="wy", name="wy")
            aw0 = int(xoffB[0, 0] - xcl0[0])  # A-blocks of slot 0
            nc.sync.dma_start(wx_t[0][:, :aw0], wxd[:, :aw0])
            nc.scalar.dma_start(xc_t[0][:, :512], xcd[0][:, :512])
            nc.sync.dma_start(wx_t[0][:, aw0:], wxd[:, aw0:int(xcl0[1])])
            nc.scalar.dma_start(xc_t[0][:, 512:], xcd[0][:, 512:])
            nc.gpsimd.dma_start(wy_t[0][:], wyd[:, :int(ycl0[1])])
            for cl in range(1, C_PER_CORE):
                nc.gpsimd.dma_start(wx_t[cl][:],
                                    wxd[:, int(xcl0[cl]):int(xcl0[cl + 1])])
                nc.gpsimd.dma_start(xc_t[cl][:], xcd[cl])
                nc.gpsimd.dma_start(wy_t[cl][:],
                                    wyd[:, int(ycl0[cl]):int(ycl0[cl + 1])])

            # ---- warmup ---------------------------------------------------
            warm_ps = pso_pool.tile([128, 1024], f32, tag="pso", name="pso")
            for _ in range(WARM_MMS):
                nc.tensor.matmul(warm_ps[:, :128], warm_sb[:], warm_sb[:],
                                 start=True, stop=True)

            # ---- per-(b,c) emission --------------------------------------
            psv_tiles = {}   # t -> [psv_jh0, psv_jh1]
            vt_tiles = {}    # t -> [vt0, vt1]
            osb_tiles = {}
            pso_tiles = {}   # (t, fp) -> tile

            def s1_unit(t, jh, pc):
                """Stage-1 matmuls for one (jh, pc)."""
                cl, b = divmod(t, 4)
                if jh == 0 and pc == 0:
                    psv_tiles[t] = [
                        psv_pool.tile([128, 1024], f32, tag="psv", name="psv")
                        for _ in range(2)]
                psv = psv_tiles[t][jh]
                xt = xc_t[cl][:, b * 512 + pc * 256 + jh * 128:
                              b * 512 + pc * 256 + jh * 128 + 128]
                for bank in range(2):
                    W0 = int(W0u[cl, bank])
                    L1 = int(L1u[cl, bank])
                    W1 = 256 - L1
                    pblk = psv[:, bank * 512:(bank + 1) * 512].rearrange(
                        "p (g c) -> p g c", g=2)
                    if pc == 0:
                        rhs = wx_t[cl][:, int(xoffA[cl, bank] - xcl0[cl]):
                                       int(xoffA[cl, bank] - xcl0[cl]) + 2 * W0]
                        nc.tensor.matmul(pblk[:, :, 0:W0], xt, rhs,
                                         start=True, stop=False)
                    else:
                        boff = int(xoffB[cl, bank] - xcl0[cl])
                        wblk = wx_t[cl][:, boff:boff + 2 * W1].rearrange(
                            "p (g w) -> p g w", g=2)
                        nc.tensor.matmul(pblk[:, :, L1:W0], xt,
                                         wblk[:, :, 0:W0 - L1],
                                         start=False, stop=True)
                        if W0 < 256:
                            nc.tensor.matmul(pblk[:, :, W0:256], xt,
                                             wblk[:, :, W0 - L1:W0 - L1 + 256 - W0],
                                             start=True, stop=True)

            def s1_copy(t, jh):
                if t not in vt_tiles:
                    vt_tiles[t] = [
                        vt_pool.tile([128, 1024], f16, tag="vt", name="vt")
                        for _ in range(2)]
                eng = nc.vector.tensor_copy if jh == 0 else nc.scalar.copy
                eng(vt_tiles[t][jh][:], psv_tiles[t][jh][:])

            def s2_unit(t, fp, ih):
                """Stage-2 matmuls for one (f-position, ih)."""
                cl, b = divmod(t, 4)
                pair = fp // 2
                if fp % 2 == 0 and ih == 0:
                    pso_tiles[(t, pair)] = pso_pool.tile([128, 1024], f32,
                                                         tag="pso", name="pso")
                pso = pso_tiles[(t, pair)]
                G0 = int(G0u[cl, fp])
                M1 = int(M1u[cl, fp])
                yo0 = int(yoff[cl, fp] - ycl0[cl])
                yo1 = yo0 + G0
                vt0, vt1 = vt_tiles[t]
                st0 = vt0[:, fp * 256 + ih * 128: fp * 256 + ih * 128 + 128]
                st1 = vt1[:, fp * 256 + ih * 128: fp * 256 + ih * 128 + 128]
                o = (fp % 2) * 512 + ih * 256
                nc.tensor.matmul(pso[:, o:o + G0], st0,
                                 wy_t[cl][:, yo0:yo0 + G0],
                                 start=True, stop=False)
                nc.tensor.matmul(pso[:, o + M1:o + G0], st1,
                                 wy_t[cl][:, yo1:yo1 + G0 - M1],
                                 start=False, stop=True)
                if G0 < 256:
                    nc.tensor.matmul(pso[:, o + G0:o + 256], st1,
                                     wy_t[cl][:, yo1 + G0 - M1:yo1 + 256 - M1],
                                     start=True, stop=True)

            def s2_copy(t, fp, eng_idx, whole_pair):
                cl, b = divmod(t, 4)
                if t not in osb_tiles:
                    osb_tiles[t] = o_pool.tile([128, 2048], f16, tag="o",
                                               name="osb")
                pair = fp // 2
                eng = nc.vector.tensor_copy if eng_idx == 0 else nc.scalar.copy
                if whole_pair:
                    dst = osb_tiles[t][:, pair * 1024:(pair + 1) * 1024]
                    eng(dst[:], pso_tiles[(t, pair)][:])
                else:
                    dst = osb_tiles[t][:, fp * 512:(fp + 1) * 512]
                    src = pso_tiles[(t, pair)][:, (fp % 2) * 512:
                                               (fp % 2) * 512 + 512]
                    eng(dst[:], src[:])

            def store(t, split):
                cl, b = divmod(t, 4)
                osb = osb_tiles[t]
                if split == 1:
                    nc.sync.dma_start(out[b, cl], osb[:])
                else:
                    stp = 2048 // split
                    for q in range(split):
                        nc.sync.dma_start(out[b, cl][:, q * stp:(q + 1) * stp],
                                          osb[:, q * stp:(q + 1) * stp])

            def emit_s1(t):
                """Full stage-1 of t as a unit list (callables)."""
                units = []
                for jh in range(2):
                    units.append(lambda t=t, jh=jh: s1_unit(t, jh, 0))
                    def u(t=t, jh=jh):
                        s1_unit(t, jh, 1)
                        s1_copy(t, jh)
                    units.append(u)
                return units

            def emit_s2(t, last=False):
                units = []
                for fp in range(FN):
                    units.append(lambda t=t, fp=fp: s2_unit(t, fp, 0))
                    def u(t=t, fp=fp, last=last):
                        s2_unit(t, fp, 1)
                        if last:
                            # per-f copies alternating engines + immediate
                            # per-f stores: shortest possible tail
                            s2_copy(t, fp, fp % 2, False)
                            cl, b = divmod(t, 4)
                            nc.sync.dma_start(
                                out[b, cl][:, fp * 512:(fp + 1) * 512],
                                osb_tiles[t][:, fp * 512:(fp + 1) * 512])
                        elif fp % 2 == 1:
                            # fused pair copy: pair0 -> vector, pair1 -> scalar
                            s2_copy(t, fp, fp // 2, True)
                            if fp == 3:
                                store(t, 1)
                    units.append(u)
                return units

            # interleave: window t runs S2(t-LAG) units with S1(t) units
            # spread between them (S2 first: S1(t)'s psum buffers recycle
            # from t-1, whose copies finish while S2(t-LAG) runs).
            for t in range(NT + LAG):
                s1u = emit_s1(t) if t < NT else []
                t2 = t - LAG
                s2u = emit_s2(t2, last=(t2 == NT - 1)) if t2 >= 0 else []
                if not s1u:
                    for u in s2u:
                        u()
                elif not s2u:
                    for u in s1u:
                        u()
                else:
                    n1, n2 = len(s1u), len(s2u)
                    k1 = 0
                    for k2 in range(n2):
                        s2u[k2]()
                        take = ((k2 + 1) * n1) // n2 - k1
                        for _ in range(take):
                            s1u[k1]()
                            k1 += 1
                    while k1 < n1:
                        s1u[k1]()
                        k1 += 1

    nc.compile()
    return nc


def _get_program(plan):
    key = _range_key(plan)
    if key not in _PROGRAM_CACHE:
        _PROGRAM_CACHE[key] = _build_program(plan)
    return _PROGRAM_CACHE[key]


def _prepare_in_maps(x, plan):
    x16 = x.astype(np.float16)
    WxT, WyT = plan["WxT"], plan["WyT"]
    perm, assign = plan["perm"], plan["assign"]
    W0u, L1u, G0u, M1u = plan["W0u"], plan["L1u"], plan["G0u"], plan["M1u"]
    xoffA, xoffB, yoff = plan["xoffA"], plan["xoffB"], plan["yoff"]
    XW, YW = plan["XW"], plan["YW"]

    in_maps = []
    for k in range(N_CORES):
        xc = np.zeros((C_PER_CORE, 128, 2048), np.float16)
        wx = np.zeros((128, XW), np.float16)
        wy = np.zeros((128, YW), np.float16)
        for cl in range(C_PER_CORE):
            ch = assign[k][cl]
            # xc[cl][p, b*512 + pc*256 + j]
            xc[cl] = (x16[:, ch].reshape(B, 2, 128, 256)
                      .transpose(2, 0, 1, 3).reshape(128, 2048))
            for bank in range(2):
                W0 = int(W0u[cl, bank])
                L1 = int(L1u[cl, bank])
                W1 = 256 - L1
                for i, fp in enumerate((2 * bank, 2 * bank + 1)):
                    f = int(perm[ch][fp])
                    a = int(xoffA[cl, bank]) + i * W0
                    wx[:, a:a + W0] = WxT[ch, f][0:128, 0:W0]
                    bo = int(xoffB[cl, bank]) + i * W1
                    wx[:, bo:bo + W1] = WxT[ch, f][128:256, L1:256]
            for fp in range(FN):
                f = int(perm[ch][fp])
                G0 = int(G0u[cl, fp])
                M1 = int(M1u[cl, fp])
                a = int(yoff[cl, fp])
                wy[:, a:a + G0] = WyT[ch, f][0:128, 0:G0]
                wy[:, a + G0:a + G0 + 256 - M1] = WyT[ch, f][128:256, M1:256]
        in_maps.append({"xc": xc, "wx": wx, "wy": wy})
    return in_maps


def run(input, x_min, x_max, y_min, y_max, trace=False):
    from concourse.bass_utils import run_bass_kernel_spmd

    x = np.asarray(input, dtype=np.float32)
    plan = _plan(np.asarray(x_min, np.float64), np.asarray(x_max, np.float64),
                 np.asarray(y_min, np.float64), np.asarray(y_max, np.float64))
    nc = _get_program(plan)
    in_maps = _prepare_in_maps(x, plan)
    res = run_bass_kernel_spmd(nc, in_maps, list(range(N_CORES)), trace=trace)

    perm, assign = plan["perm"], plan["assign"]
    full = np.empty((B, C * FN, 256, 256), np.float32)
    for k in range(N_CORES):
        o = res.results[k]["out"].astype(np.float32)
        # o[b, cl, p, fp*512 + ih*256 + jo]
        o = o.reshape(B, C_PER_CORE, 128, FN, 2, 256)
        o = o.transpose(0, 1, 3, 4, 2, 5)  # [b, cl, fp, ih, p, jo]
        for cl in range(C_PER_CORE):
            ch = assign[k][cl]
            idx = ch * FN + perm[ch]  # output channel per f-position
            full[:, idx] = o[:, cl].reshape(B, FN, 256, 256)
    return full, res


def kernel(input, x_min, x_max, y_min, y_max):
    full, _ = run(input, x_min, x_max, y_min, y_max)
    return full
